# revision 11
# baseline (speedup 1.0000x reference)
"""CrossAttentionFusion forward on 8 Trainium2 NeuronCores (pure data parallel).

Math folded on host (seq-len-1 MHA == two chained linears):
  d_att = micro @ A_dm + c_dm,  A_dm = Wv_dm.T @ Wout_dm.T
  m_att = drug  @ A_md + c_md
  u = drug + d_att ; w = micro + m_att
  xu = (u - mu)/sd ; xw likewise        (LN affine folded into W1)
  h1 = gelu([xu, xw] @ W1f + b1f),  W1f = (ffn_w1 * g_cat).T
  h2 = h1 @ W2f + b2,               W2f = ffn_w2.T
  out = ((h2 - mu)/sd) * g_out + b_out

Device layout: activations feature-major [feat(partition), batch(free)];
batch sharded across 8 cores, tiles of NB=512 columns.

Perf structure (vs the naive per-tile loop):
  - 4-deep software pipeline: iteration i emits attn(i), LN-stats(i-1),
    ffn2(i-2), out-LN(i-3) so the PE instruction stream never waits on
    vector/scalar work (keeps the HAM clock-gate at 8/8 = 2.4 GHz).
  - LN sum+sumsq fused into one fp8 DoubleRow matmul per feature slab
    (pair dim = [x, x^2]); stats-only fp8 error is O(3.6%/sqrt(384)).
  - rstd via DVE fast-inverse-sqrt (bit trick + Newton) so the scalar
    engine only ever runs the gelu table set (no ACT_TABLE_LOAD thrash).
All main matmuls stay bf16 with fp32 PSUM accumulation.
"""

import sys

if "/opt/trn_rl_repo" not in sys.path:
    sys.path.insert(0, "/opt/trn_rl_repo")

from contextlib import ExitStack

import ml_dtypes
import numpy as np

import concourse.bass as bass  # noqa: F401  (registers mybir lowering hooks)
import concourse.tile as tile
from concourse import bacc, mybir
from concourse.bass import ts
from concourse.bass_utils import run_bass_kernel_spmd

F32 = mybir.dt.float32
BF16 = mybir.dt.bfloat16
F8 = mybir.dt.float8e4
I32 = mybir.dt.int32
ACT = mybir.ActivationFunctionType
ALU = mybir.AluOpType
DR = mybir.MatmulPerfMode.DoubleRow

P = 128
D = 384
KD = D // P          # 3
DH = 2 * D           # 768
KH = DH // P         # 6
DF = 4 * D           # 1536
KF = DF // P         # 12
EPS = 1e-5
N_CORES = 8
B_FULL = 65536
BC = B_FULL // N_CORES   # 8192 rows per core
NB = 512                 # batch columns per on-chip tile

MAGIC = 0x5F3759E0       # fisr magic + 1 (used as ~(i>>1) + MAGIC)

_NC_CACHE = {}
LAST_RESULTS = None      # BassKernelResults of the most recent kernel() call


def _build_nc(bc, nb, flags):
    use_c_dm, use_c_md, use_b1, use_b2, use_affine = flags
    nt = bc // nb
    nc = bacc.Bacc("TRN2", target_bir_lowering=False, debug=False,
                   num_devices=N_CORES)

    xd_d = nc.dram_tensor("xd", [D, bc], BF16, kind="ExternalInput")
    xm_d = nc.dram_tensor("xm", [D, bc], BF16, kind="ExternalInput")
    a_dm_d = nc.dram_tensor("a_dm", [D, D], BF16, kind="ExternalInput")
    a_md_d = nc.dram_tensor("a_md", [D, D], BF16, kind="ExternalInput")
    w1_d = nc.dram_tensor("w1", [DH, DF], BF16, kind="ExternalInput")
    w2_d = nc.dram_tensor("w2", [DF, D], BF16, kind="ExternalInput")
    c_dm_d = nc.dram_tensor("c_dm", [D], F32, kind="ExternalInput") if use_c_dm else None
    c_md_d = nc.dram_tensor("c_md", [D], F32, kind="ExternalInput") if use_c_md else None
    b1_d = nc.dram_tensor("b1", [DF], F32, kind="ExternalInput") if use_b1 else None
    b2_d = nc.dram_tensor("b2", [D], F32, kind="ExternalInput") if use_b2 else None
    g_o_d = nc.dram_tensor("g_o", [D], F32, kind="ExternalInput") if use_affine else None
    b_o_d = nc.dram_tensor("b_o", [D], F32, kind="ExternalInput") if use_affine else None
    o_d = nc.dram_tensor("o", [D, bc], F32, kind="ExternalOutput")

    xd_r = xd_d.ap().rearrange("(k p) n -> p k n", p=P)
    xm_r = xm_d.ap().rearrange("(k p) n -> p k n", p=P)
    o_r = o_d.ap().rearrange("(k p) n -> p k n", p=P)

    with tile.TileContext(nc) as tc, ExitStack() as ctx:
        wp = ctx.enter_context(tc.tile_pool(name="wts", bufs=1))
        xp = ctx.enter_context(tc.tile_pool(name="x", bufs=3))
        up = ctx.enter_context(tc.tile_pool(name="u", bufs=2))
        s8p = ctx.enter_context(tc.tile_pool(name="s8", bufs=2))
        xhp = ctx.enter_context(tc.tile_pool(name="xh", bufs=2))
        h1p = ctx.enter_context(tc.tile_pool(name="h1", bufs=2))
        h2p = ctx.enter_context(tc.tile_pool(name="h2", bufs=2))
        op_ = ctx.enter_context(tc.tile_pool(name="o", bufs=2))
        stp = ctx.enter_context(tc.tile_pool(name="st", bufs=2))
        pmm = ctx.enter_context(tc.tile_pool(name="pmm", bufs=4, space="PSUM"))
        pst = ctx.enter_context(tc.tile_pool(name="pst", bufs=2, space="PSUM"))
        pbc = ctx.enter_context(tc.tile_pool(name="pbc", bufs=2, space="PSUM"))

        a_dm_sb = wp.tile([P, KD, D], BF16)
        nc.gpsimd.dma_start(a_dm_sb[:], a_dm_d.ap().rearrange("(k p) m -> p k m", p=P))
        a_md_sb = wp.tile([P, KD, D], BF16)
        nc.gpsimd.dma_start(a_md_sb[:], a_md_d.ap().rearrange("(k p) m -> p k m", p=P))
        w1_sb = wp.tile([P, KH, DF], BF16)
        nc.gpsimd.dma_start(w1_sb[:], w1_d.ap().rearrange("(k p) m -> p k m", p=P))
        w2_sb = wp.tile([P, KF, D], BF16)
        nc.gpsimd.dma_start(w2_sb[:], w2_d.ap().rearrange("(k p) m -> p k m", p=P))

        # DoubleRow stats weights: pair dim = (x, x^2); sum x lands on out
        # partition 0, sum x^2 on out partition 32 (engine PSUM reads must
        # start on a quadrant base). Pair-dim stride must be 16-aligned
        # (s3_lw dual-fp8 restriction), hence the 48-wide stationary.
        SQP = 32
        WPW = 48
        wpair = wp.tile([P, 2, WPW], F8)
        nc.vector.memset(wpair[:], 0.0)
        nc.vector.memset(wpair[:, 0, 0:1], 1.0)
        nc.vector.memset(wpair[:, 1, SQP:SQP + 1], 1.0)
        ones_bc = wp.tile([1, P], BF16)
        nc.vector.memset(ones_bc[:], 1.0)

        def vec_const(dram, nk, tag):
            t = wp.tile([P, nk], F32, tag=tag)
            nc.gpsimd.dma_start(t[:], dram.ap().rearrange("(k p) -> p k", p=P))
            return t

        c_dm_sb = vec_const(c_dm_d, KD, "c_dm") if use_c_dm else None
        c_md_sb = vec_const(c_md_d, KD, "c_md") if use_c_md else None
        b1_sb = vec_const(b1_d, KF, "b1") if use_b1 else None
        b2_sb = vec_const(b2_d, KD, "b2") if use_b2 else None
        g_o_sb = vec_const(g_o_d, KD, "g_o") if use_affine else None
        b_o_sb = vec_const(b_o_d, KD, "b_o") if use_affine else None

        state = {}

        def emit_load(j):
            sl = slice(j * NB, (j + 1) * NB)
            xd = xp.tile([P, KD, NB], BF16, tag="xd")
            nc.sync.dma_start(xd[:], xd_r[:, :, sl])
            xm = xp.tile([P, KD, NB], BF16, tag="xm")
            nc.sync.dma_start(xm[:], xm_r[:, :, sl])
            state[j] = {"xd": xd, "xm": xm}

        # ---- per-stage emitters (PE / DVE / scalar streams interleave by
        # global emission order; each engine runs its own subsequence
        # in-order) ----

        def a_mm(j):  # PE x18
            s = state[j]
            aps = []
            for a_sb, rhs in ((a_dm_sb, s["xm"]), (a_md_sb, s["xd"])):
                for m in range(KD):
                    ps = pmm.tile([P, NB], F32, tag="mm")
                    for k in range(KD):
                        nc.tensor.matmul(ps[:], a_sb[:, k, ts(m, P)],
                                         rhs[:, k, :],
                                         start=(k == 0), stop=(k == KD - 1))
                    aps.append(ps)
            s["aps"] = aps

        def adds(j):  # DVE x6: u = attn_psum + residual
            s = state[j]
            u = up.tile([P, KD, NB], BF16, tag="u")
            w = up.tile([P, KD, NB], BF16, tag="w")
            for m in range(KD):
                nc.vector.tensor_add(u[:, m, :], s["aps"][m][:], s["xd"][:, m, :])
                if use_c_dm:
                    nc.vector.tensor_scalar_add(u[:, m, :], u[:, m, :],
                                                c_dm_sb[:, m:m + 1])
            for m in range(KD):
                nc.vector.tensor_add(w[:, m, :], s["aps"][KD + m][:],
                                     s["xm"][:, m, :])
                if use_c_md:
                    nc.vector.tensor_scalar_add(w[:, m, :], w[:, m, :],
                                                c_md_sb[:, m:m + 1])
            s["u"], s["w"] = u, w
            del s["aps"]

        def sq8(j):  # scalar x6: pair slot 1 = x^2 (fp8)
            s = state[j]
            us8 = s8p.tile([P, KD, 2, NB], F8, tag="us8")
            ws8 = s8p.tile([P, KD, 2, NB], F8, tag="ws8")
            for k in range(KD):
                nc.scalar.activation(us8[:, k, 1, :], s["u"][:, k, :], ACT.Square)
            for k in range(KD):
                nc.scalar.activation(ws8[:, k, 1, :], s["w"][:, k, :], ACT.Square)
            s["us8"], s["ws8"] = us8, ws8

        def u8(j):  # DVE x6: pair slot 0 = x (fp8)
            s = state[j]
            for k in range(KD):
                nc.vector.tensor_copy(s["us8"][:, k, 0, :], s["u"][:, k, :])
            for k in range(KD):
                nc.vector.tensor_copy(s["ws8"][:, k, 0, :], s["w"][:, k, :])

        def stats_mm(ps, x8):  # PE: DR matmuls accumulate [sum x; sum x^2]
            for k in range(KD):
                nc.tensor.matmul(ps[0:WPW, :], wpair[:, :, :],
                                 x8[:, k, :, :],
                                 start=(k == 0), stop=(k == KD - 1),
                                 perf_mode=DR)

        def s_mm(j):  # PE x6
            s = state[j]
            psu = pst.tile([P, NB], F32, tag="st")
            stats_mm(psu, s["us8"])
            psw = pst.tile([P, NB], F32, tag="st")
            stats_mm(psw, s["ws8"])
            s["psu"], s["psw"] = psu, psw

        def chain(j, ps, pref):  # DVE x~16: mu16/inv16 from [sum; sumsq]
            tg = "cho" if pref == "o" else "ch"
            bn = 1 if pref == "o" else 2
            t = stp.tile([1, 6, NB], F32, tag=tg, name="cht", bufs=bn)
            t16 = stp.tile([1, 2, NB], BF16, tag=tg + "16", name="cht16",
                           bufs=bn)
            mu, mu2, av, y0, n1, y1 = (t[:, k, :] for k in range(6))
            mu16, inv16 = t16[:, 0, :], t16[:, 1, :]
            nc.vector.tensor_scalar_mul(mu, ps[0:1, :], 1.0 / D)
            nc.vector.tensor_copy(mu16, mu)
            nc.vector.tensor_mul(mu2, mu, mu)
            nc.vector.tensor_scalar(av, ps[SQP:SQP + 1, :], 1.0 / D, EPS,
                                    ALU.mult, ALU.add)
            nc.vector.tensor_sub(av, av, mu2)
            # fast inverse sqrt: y0 = bits(~(bits(v)>>1) + MAGIC), 2 Newton
            nc.vector.tensor_scalar(y0.bitcast(I32), av.bitcast(I32),
                                    1, -1, ALU.arith_shift_right,
                                    ALU.bitwise_xor)
            nc.vector.tensor_scalar_add(y0.bitcast(I32), y0.bitcast(I32),
                                        MAGIC)
            nc.vector.tensor_mul(n1, y0, y0)
            nc.vector.tensor_mul(n1, n1, av)
            nc.vector.tensor_scalar(n1, n1, -0.5, 1.5, ALU.mult, ALU.add)
            nc.vector.tensor_mul(y1, y0, n1)
            nc.vector.tensor_mul(n1, y1, y1)
            nc.vector.tensor_mul(n1, n1, av)
            nc.vector.tensor_scalar(n1, n1, -0.5, 1.5, ALU.mult, ALU.add)
            nc.vector.tensor_mul(y1, y1, n1)
            nc.vector.tensor_copy(inv16, y1)
            state[j][pref + "mu16"] = mu16
            state[j][pref + "inv16"] = inv16

        def bcast(j, pref):  # PE x2
            s = state[j]
            mu_ps = pbc.tile([P, NB], F32, tag="bc")
            nc.tensor.matmul(mu_ps[:], ones_bc[:], s[pref + "mu16"][:],
                             start=True, stop=True)
            inv_ps = pbc.tile([P, NB], F32, tag="bc")
            nc.tensor.matmul(inv_ps[:], ones_bc[:], s[pref + "inv16"][:],
                             start=True, stop=True)
            s[pref + "mu_ps"], s[pref + "inv_ps"] = mu_ps, inv_ps

        def xh_half(j, pref):  # DVE x6: xh slab = (x - mu) * inv
            s = state[j]
            if "xh" not in s:
                s["xh"] = xhp.tile([P, KH, NB], BF16, tag="xh", name="xh")
            x = s["u"] if pref == "u" else s["w"]
            base = 0 if pref == "u" else KD
            mu_ps, inv_ps = s[pref + "mu_ps"], s[pref + "inv_ps"]
            for k in range(KD):
                nc.vector.tensor_sub(x[:, k, :], x[:, k, :], mu_ps[:])
            for k in range(KD):
                nc.vector.tensor_mul(s["xh"][:, base + k, :], x[:, k, :],
                                     inv_ps[:])

        def f1_half(j, ms):  # PE x36 + scalar gelu x6
            s = state[j]
            if "h1" not in s:
                s["h1"] = h1p.tile([P, KF, NB], BF16, tag="h1", name="h1")
            for m in ms:
                ps = pmm.tile([P, NB], F32, tag="mm")
                for k in range(KH):
                    nc.tensor.matmul(ps[:], w1_sb[:, k, ts(m, P)],
                                     s["xh"][:, k, :],
                                     start=(k == 0), stop=(k == KH - 1))
                if use_b1:
                    nc.scalar.activation(s["h1"][:, m, :], ps[:], ACT.Gelu,
                                         bias=b1_sb[:, m:m + 1])
                else:
                    nc.scalar.activation(s["h1"][:, m, :], ps[:], ACT.Gelu)

        def f2_group(j, m):  # PE x12
            s = state[j]
            ps = pmm.tile([P, NB], F32, tag="mm")
            for k in range(KF):
                nc.tensor.matmul(ps[:], w2_sb[:, k, ts(m, P)], s["h1"][:, k, :],
                                 start=(k == 0), stop=(k == KF - 1))
            s.setdefault("f2ps", []).append(ps)

        def h2cp(j):  # DVE x6: h2 bf16 (+b2) and fp8 pair slot 0
            s = state[j]
            h2 = h2p.tile([P, KD, NB], BF16, tag="h2")
            for m in range(KD):
                if use_b2:
                    nc.vector.tensor_scalar_add(h2[:, m, :], s["f2ps"][m][:],
                                                b2_sb[:, m:m + 1])
                else:
                    nc.vector.tensor_copy(h2[:, m, :], s["f2ps"][m][:])
            h28 = s8p.tile([P, KD, 2, NB], F8, tag="h28")
            for m in range(KD):
                nc.vector.tensor_copy(h28[:, m, 0, :], h2[:, m, :])
            s["h2"], s["h28"] = h2, h28
            del s["f2ps"]

        def sqh8(j):  # scalar x3
            s = state[j]
            for m in range(KD):
                nc.scalar.activation(s["h28"][:, m, 1, :], s["h2"][:, m, :],
                                     ACT.Square)

        def s2_mm(j):  # PE x3
            s = state[j]
            pso = pst.tile([P, NB], F32, tag="st")
            stats_mm(pso, s["h28"])
            s["pso"] = pso

        def oap(j):  # DVE x6 + store DMA
            s = state[j]
            o = op_.tile([P, KD, NB], F32, tag="o")
            mu_ps, inv_ps = s["omu_ps"], s["oinv_ps"]
            for k in range(KD):
                nc.vector.tensor_sub(s["h2"][:, k, :], s["h2"][:, k, :],
                                     mu_ps[:])
            for k in range(KD):
                nc.vector.tensor_mul(o[:, k, :], s["h2"][:, k, :], inv_ps[:])
                if use_affine:
                    nc.vector.tensor_scalar(o[:, k, :], o[:, k, :],
                                            g_o_sb[:, k:k + 1],
                                            b_o_sb[:, k:k + 1],
                                            ALU.mult, ALU.add)
            sl = slice(j * NB, (j + 1) * NB)
            nc.sync.dma_start(o_r[:, :, sl], o[:])

        # ---- pipelined emission ----
        emit_load(0)
        if nt > 1:
            emit_load(1)
        for i in range(nt + 3):
            j0, j1, j2, j3 = i, i - 1, i - 2, i - 3
            e0 = j0 < nt
            e1 = 0 <= j1 < nt
            e2 = 0 <= j2 < nt
            e3 = 0 <= j3 < nt
            if e0:
                a_mm(j0)          # PE
                adds(j0)          # DVE
            if e1:
                s_mm(j1)          # PE
                chain(j1, state[j1]["psu"], "u")   # DVE
                chain(j1, state[j1]["psw"], "w")   # DVE
            if e0:
                sq8(j0)           # scalar
            if e3:
                s2_mm(j3)         # PE
                chain(j3, state[j3]["pso"], "o")   # DVE
            if e2:
                f2_group(j2, 0)   # PE
            if e1:
                bcast(j1, "u")    # PE
                xh_half(j1, "u")  # DVE
            if e2:
                f2_group(j2, 1)   # PE
            if e1:
                bcast(j1, "w")    # PE
                xh_half(j1, "w")  # DVE
            if e2:
                f2_group(j2, 2)   # PE
                h2cp(j2)          # DVE
                sqh8(j2)          # scalar
            if e1:
                f1_half(j1, range(0, 6))    # PE + scalar
            if e3:
                bcast(j3, "o")    # PE
            if e0:
                u8(j0)            # DVE
            if e3:
                oap(j3)           # DVE + DMA store
            if e1:
                f1_half(j1, range(6, 12))   # PE + scalar
            if i + 2 < nt:
                emit_load(i + 2)  # DMA prefetch
            if e3:
                del state[j3]

    nc.compile()
    return nc


def kernel(**inputs) -> np.ndarray:
    global LAST_RESULTS
    f = lambda k: np.asarray(inputs[k], np.float32)

    drug = f("drug_emb")
    micro = f("micro_emb")
    b = drug.shape[0]
    bc = b // N_CORES
    assert b % (N_CORES * NB) == 0

    # ---- host-side weight folding ----
    wv_dm, bv_dm = f("dm_in_w")[2 * D:], f("dm_in_b")[2 * D:]
    wv_md, bv_md = f("md_in_w")[2 * D:], f("md_in_b")[2 * D:]
    a_dm = np.ascontiguousarray(wv_dm.T @ f("dm_out_w").T).astype(ml_dtypes.bfloat16)
    c_dm = bv_dm @ f("dm_out_w").T + f("dm_out_b")
    a_md = np.ascontiguousarray(wv_md.T @ f("md_out_w").T).astype(ml_dtypes.bfloat16)
    c_md = bv_md @ f("md_out_w").T + f("md_out_b")
    g_cat = np.concatenate([f("norm_d_g"), f("norm_m_g")])
    b_cat = np.concatenate([f("norm_d_b"), f("norm_m_b")])
    w1f = np.ascontiguousarray((f("ffn_w1") * g_cat[None, :]).T).astype(ml_dtypes.bfloat16)
    b1f = f("ffn_b1") + b_cat @ f("ffn_w1").T
    w2f = np.ascontiguousarray(f("ffn_w2").T).astype(ml_dtypes.bfloat16)
    b2 = f("ffn_b2")
    g_o, b_o = f("norm_out_g"), f("norm_out_b")

    flags = (bool(np.any(c_dm)), bool(np.any(c_md)), bool(np.any(b1f)),
             bool(np.any(b2)), bool(np.any(g_o != 1.0) or np.any(b_o)))

    key = (bc, NB, flags)
    if key not in _NC_CACHE:
        _NC_CACHE[key] = _build_nc(bc, NB, flags)
    nc = _NC_CACHE[key]

    in_maps = []
    for c in range(N_CORES):
        sl = slice(c * bc, (c + 1) * bc)
        m = {
            "xd": np.ascontiguousarray(drug[sl].T).astype(ml_dtypes.bfloat16),
            "xm": np.ascontiguousarray(micro[sl].T).astype(ml_dtypes.bfloat16),
            "a_dm": a_dm, "a_md": a_md, "w1": w1f, "w2": w2f,
        }
        if flags[0]:
            m["c_dm"] = c_dm
        if flags[1]:
            m["c_md"] = c_md
        if flags[2]:
            m["b1"] = b1f
        if flags[3]:
            m["b2"] = b2
        if flags[4]:
            m["g_o"] = g_o
            m["b_o"] = b_o
        in_maps.append(m)

    res = run_bass_kernel_spmd(nc, in_maps, list(range(N_CORES)))
    LAST_RESULTS = res

    out = np.empty((b, D), np.float32)
    for c in range(N_CORES):
        out[c * bc:(c + 1) * bc] = res.results[c]["o"].T
    return out
